# revision 14
# baseline (speedup 1.0000x reference)
"""Multi-head causal attention (B=4, S=2048, D=1024, H=16) on 8 trn2 NeuronCores.

Sharding: data-parallel over batch (4) x tensor-parallel over heads (2 groups
of 8 heads).  Core c handles batch c//2, head-group c%2.  Each core computes
its 512-wide slice of Q/K/V, causal attention for its 8 heads, and a partial
out-projection (row-parallel Wo).  The host sums the two partials per batch
and adds the bias (the "all-reduce" of the row-parallel out_proj).

Kernel layout notes (per core):
 - x arrives pre-transposed from host as xt [1024, 2048] so the contraction
   dim (d_in) is on partitions for all projection matmuls.
 - Q^T, K^T stored [d'=128 (2 heads), s] in bf16: directly usable as
   scores-matmul operands (S^T[k,q] = K^T_tile.T @ Q^T) with d on partitions.
 - V stored naturally [s, d'] with a ones-column appended per head (65-wide
   head slots) so the ctx matmul also produces the softmax denominators.
 - Scores are computed transposed (S^T: k on partitions, q free).  Softmax
   needs no max-stabilization (scores ~ N(0,1) after the 1/8 scale), so
   exp is a single ScalarE pass PSUM->SBUF; causal masking is a bf16
   multiply with one of 4 precomputed staircase masks; the denominator
   comes from the V ones-column; normalization uses a K=1 broadcast matmul.
 - ctx^T [d', q] feeds the out-projection as the stationary operand, and the
   output lands naturally as [q, e] for a clean DMA out.
"""

import numpy as np

import concourse.bacc as bacc
import concourse.bass as bass
import concourse.mybir as mybir
from concourse import tile
from concourse.bass_utils import run_bass_kernel_spmd

F32 = mybir.dt.float32
F32R = mybir.dt.float32r
BF16 = mybir.dt.bfloat16
EXP = mybir.ActivationFunctionType.Exp

B, S, DIN, DOUT, H = 4, 2048, 1024, 1024, 16
NCORES = 8
DG = 512          # d_out slice per core (8 heads)
NH = 8            # heads per core
HD = 64
NKT = DIN // 128  # 8 contraction tiles for projections
NQB = S // 512    # 4 q blocks of 512
NKB = S // 128    # 16 k blocks of 128
NDB = DG // 128   # 4 d'-blocks of 128 (2 heads each)

LAST_EXEC_TIME_NS = None


def build_nc():
    nc = bacc.Bacc()
    xt = nc.dram_tensor("xt", [DIN, S], BF16, kind="ExternalInput")
    wq = nc.dram_tensor("wq", [DIN, DG], BF16, kind="ExternalInput")
    wk = nc.dram_tensor("wk", [DIN, DG], BF16, kind="ExternalInput")
    wv = nc.dram_tensor("wv", [DIN, DG], BF16, kind="ExternalInput")
    wo = nc.dram_tensor("wo", [DG, DOUT], BF16, kind="ExternalInput")
    out = nc.dram_tensor("out", [S, DOUT], F32, kind="ExternalOutput")

    with tile.TileContext(nc) as tc:
        with (
            tc.tile_pool(name="persist", bufs=1) as persist,
            tc.tile_pool(name="xt", bufs=2) as xt_pool,
            tc.tile_pool(name="eb", bufs=3) as e_pool,
            tc.tile_pool(name="rp", bufs=2) as r_pool,
            tc.tile_pool(name="ob", bufs=3) as o_pool,
            tc.tile_pool(name="psA", bufs=2, space="PSUM") as psA,
            tc.tile_pool(name="psC", bufs=2, space="PSUM") as psC,
        ):
            # ---- persistent SBUF tensors ----
            wq_sb = persist.tile([128, NKT, DG], BF16)
            wk_sb = persist.tile([128, NKT, DG], BF16)
            wv_sb = persist.tile([128, NKT, DG], BF16)
            wo_sb = persist.tile([128, NDB, DOUT], BF16)
            qt_sb = persist.tile([128, NDB, S], BF16)
            kt_sb = persist.tile([128, NDB, S], BF16)
            v_sb = persist.tile([128, NKB, NH, HD + 1], BF16)
            ct_sb = persist.tile([128, NDB, S], BF16)
            mask_sb = persist.tile([128, 4, 512], BF16)
            ones_sb = persist.tile([1, 64], BF16)

            # ---- one-time setup ----
            nc.vector.memset(ones_sb[:], 1.0)
            nc.vector.memset(v_sb[:, :, :, HD : HD + 1], 1.0)
            for d in range(4):
                nc.vector.memset(mask_sb[:, d, :], 1.0)
                # keep elements where q_local - k_local - 128*d >= 0
                nc.gpsimd.affine_select(
                    out=mask_sb[:, d, :],
                    in_=mask_sb[:, d, :],
                    pattern=[[1, 512]],
                    base=-128 * d,
                    channel_multiplier=-1,
                    compare_op=mybir.AluOpType.is_ge,
                    fill=0.0,
                )

            # ---- weight loads ----
            nc.sync.dma_start(
                out=wq_sb[:], in_=wq.rearrange("(kt p) d -> p kt d", p=128)
            )
            nc.sync.dma_start(
                out=wk_sb[:], in_=wk.rearrange("(kt p) d -> p kt d", p=128)
            )
            nc.sync.dma_start(
                out=wv_sb[:], in_=wv.rearrange("(kt p) d -> p kt d", p=128)
            )
            nc.sync.dma_start(
                out=wo_sb[:], in_=wo.rearrange("(t p) e -> p t e", p=128)
            )

            # ---- phase A: projections ----
            xt_r = xt.rearrange("(kt p) s -> p kt s", p=128)
            for n in range(NQB):
                xt_t = xt_pool.tile([128, NKT, 512], BF16, tag="xt")
                nc.sync.dma_start(out=xt_t[:], in_=xt_r[:, :, n * 512 : (n + 1) * 512])
                # Q^T and K^T: [d'-block 128, s 512] tiles, W chunk stationary
                for w_sb, dst in ((wq_sb, qt_sb), (wk_sb, kt_sb)):
                    for mp in range(2):  # pairs of d'-blocks share one psum tile
                        ps = psA.tile([128, 1024], F32)
                        for m01 in range(2):
                            m = mp * 2 + m01
                            for kt in range(NKT):
                                nc.tensor.matmul(
                                    ps[:, m01 * 512 : (m01 + 1) * 512],
                                    lhsT=w_sb[:, kt, m * 128 : (m + 1) * 128],
                                    rhs=xt_t[:, kt, :],
                                    start=(kt == 0),
                                    stop=(kt == NKT - 1),
                                )
                        nc.vector.tensor_copy(
                            dst[:, mp * 2 : mp * 2 + 2, n * 512 : (n + 1) * 512],
                            ps.rearrange("p (m s) -> p m s", m=2),
                        )
                # V: natural [s-sub 128, d' 512], x^T chunk stationary
                for sp in range(2):
                    ps = psA.tile([128, 1024], F32)
                    for s01 in range(2):
                        ss = sp * 2 + s01
                        for kt in range(NKT):
                            nc.tensor.matmul(
                                ps[:, s01 * 512 : (s01 + 1) * 512],
                                lhsT=xt_t[:, kt, ss * 128 : (ss + 1) * 128],
                                rhs=wv_sb[:, kt, :],
                                start=(kt == 0),
                                stop=(kt == NKT - 1),
                            )
                    gss = n * 4 + sp * 2
                    nc.vector.tensor_copy(
                        v_sb[:, gss : gss + 2, :, 0:HD],
                        ps.rearrange("p (u h e) -> p u h e", u=2, e=HD),
                    )

            # ---- phase B: attention ----
            for h in range(NH):
                dblk, poff = h // 2, (h % 2) * 64
                for j in range(NQB):
                    nkb = 4 * j + 4  # causal: k-blocks 0 .. 4j+3
                    pc = psC.tile([65, 512], F32)
                    for ib in range(nkb // 2):
                        ps = psA.tile([128, 1024], F32)
                        for t in range(2):
                            i = 2 * ib + t
                            nc.tensor.matmul(
                                ps[:, t * 512 : (t + 1) * 512],
                                lhsT=kt_sb[
                                    poff : poff + 64, dblk, i * 128 : (i + 1) * 128
                                ],
                                rhs=qt_sb[
                                    poff : poff + 64, dblk, j * 512 : (j + 1) * 512
                                ],
                                start=True,
                                stop=True,
                            )
                        eb = e_pool.tile([128, 1024], BF16)
                        nc.scalar.activation(eb[:], ps[:], EXP, scale=0.125)
                        for t in range(2):
                            i = 2 * ib + t
                            d = i - 4 * j
                            if d >= 0:
                                nc.vector.tensor_mul(
                                    eb[:, t * 512 : (t + 1) * 512],
                                    eb[:, t * 512 : (t + 1) * 512],
                                    mask_sb[:, d, :],
                                )
                            nc.tensor.matmul(
                                pc[:],
                                lhsT=v_sb[:, i, h, :],
                                rhs=eb[:, t * 512 : (t + 1) * 512],
                                start=(i == 0),
                                stop=(i == nkb - 1),
                            )
                    # normalize: ct = pc[0:64] * (1/denominator) broadcast over d
                    rc = r_pool.tile([1, 512], BF16, tag="rc")
                    with nc.allow_low_precision(reason="bf16 softmax denom"):
                        nc.vector.reciprocal(rc[:], pc[64:65, :])
                    pb = psA.tile([64, 512], F32, tag="psA")
                    nc.tensor.matmul(
                        pb[:],
                        lhsT=ones_sb[:],
                        rhs=rc[:],
                        start=True,
                        stop=True,
                    )
                    rb = r_pool.tile([64, 512], F32, tag="rb")
                    nc.vector.tensor_copy(rb[:], pb[:])
                    nc.vector.tensor_mul(
                        ct_sb[poff : poff + 64, dblk, j * 512 : (j + 1) * 512],
                        pc[0:64, :],
                        rb[:],
                    )

            # ---- phase C: out-projection ----
            for qq in range(S // 128):
                for e2 in range(2):
                    po = psA.tile([128, 512], F32, tag="psA")
                    for p in range(NDB):
                        nc.tensor.matmul(
                            po[:],
                            lhsT=ct_sb[:, p, qq * 128 : (qq + 1) * 128],
                            rhs=wo_sb[:, p, e2 * 512 : (e2 + 1) * 512],
                            start=(p == 0),
                            stop=(p == NDB - 1),
                        )
                    ob = o_pool.tile([128, 512], F32)
                    nc.vector.tensor_copy(ob[:], po[:])
                    nc.sync.dma_start(
                        out=out[qq * 128 : (qq + 1) * 128, e2 * 512 : (e2 + 1) * 512],
                        in_=ob[:],
                    )
    nc.compile()
    return nc


_NC_CACHE = None


def _get_nc():
    global _NC_CACHE
    if _NC_CACHE is None:
        _NC_CACHE = build_nc()
    return _NC_CACHE


NP_BF16 = mybir.dt.np(BF16)


def make_in_maps(x, Wq, Wk, Wv, Wo):
    x = np.asarray(x, dtype=np.float32).astype(NP_BF16)
    Wq = np.asarray(Wq, dtype=np.float32).astype(NP_BF16)
    Wk = np.asarray(Wk, dtype=np.float32).astype(NP_BF16)
    Wv = np.asarray(Wv, dtype=np.float32).astype(NP_BF16)
    Wo = np.asarray(Wo, dtype=np.float32).astype(NP_BF16)
    in_maps = []
    for c in range(NCORES):
        b, g = c // 2, c % 2
        sl = slice(g * DG, (g + 1) * DG)
        in_maps.append(
            {
                "xt": np.ascontiguousarray(x[b].T),
                "wq": np.ascontiguousarray(Wq[:, sl]),
                "wk": np.ascontiguousarray(Wk[:, sl]),
                "wv": np.ascontiguousarray(Wv[:, sl]),
                "wo": np.ascontiguousarray(Wo[sl, :]),
            }
        )
    return in_maps


def _install_ntff_hook():
    """Shim antenv.axon_hooks (absent in this image) so trace=True works."""
    import sys
    import types

    try:
        import antenv.axon_hooks  # noqa: F401

        return
    except ImportError:
        pass
    try:
        import antenv
        from trn_agent_boot.trn_boot import _ntff_profile_via_ctypes

        hook = _ntff_profile_via_ctypes("/opt/axon/libaxon_pjrt.so")
        mod = types.ModuleType("antenv.axon_hooks")
        mod._hook = hook
        mod.get_axon_ntff_profile_hook = lambda: mod._hook
        mod.set_axon_ntff_profile_hook = lambda h: setattr(mod, "_hook", h)
        sys.modules["antenv.axon_hooks"] = mod
        antenv.axon_hooks = mod
    except Exception as e:  # degrade to no-trace
        print("ntff hook shim failed:", e)


def kernel(x, Wq, Wk, Wv, Wo, bo, _trace=False):
    global LAST_EXEC_TIME_NS
    if _trace:
        _install_ntff_hook()
    bo = np.asarray(bo, dtype=np.float32)
    nc = _get_nc()
    in_maps = make_in_maps(x, Wq, Wk, Wv, Wo)
    res = run_bass_kernel_spmd(nc, in_maps, list(range(NCORES)), trace=_trace)
    LAST_EXEC_TIME_NS = res.exec_time_ns
    out = np.empty((B, S, DOUT), dtype=np.float32)
    for b in range(B):
        out[b] = res.results[2 * b]["out"] + res.results[2 * b + 1]["out"] + bo
    return out
